# revision 1
# baseline (speedup 1.0000x reference)
"""Trainium2 Bass kernel for single-token MoE routing (nn_MixtureOfExperts_v2).

Problem:
    x [2304]; enc_top [256, 2304]; W_down [256, 64, 2304]; encoder_weights
    [256, 512, 64].
    codes = relu_offset(enc_top @ x)           (slope 0.0, offset 1/48)
    top4 values/indices of codes
    per selected expert i (gate v):
        s = W_down[i] @ x                      [64]
        c = relu_offset(E[i] @ s, slope 0.01)  [512]
        d = E[i]^T @ c                         [64]
        recon += W_down[i]^T @ d               [2304]
        recon += v * enc_top[i]
    output = recon                             [2304]

Distribution (8 cores, no collectives):
    Every core loads a replicated bf16 transposed copy of enc_top, computes
    all 256 codes on the PE, and runs top-4 on the vector engine
    (max_with_indices), so all cores agree on the routing.  Core c then
    processes selected slot (c % 4) alone: it gathers that expert's weights
    from a per-core table in HBM with one indirect DMA and runs the expert
    pipeline.  Cores c and c+4 process the same slot but emit complementary
    halves of the 2304-dim reconstruction (the per-core tables are built
    with the core's half of the input-dim chunks first, so the program is
    identical across cores - pure SPMD with per-core constants).  The host
    sums the 8 partial outputs (the cross-core reduction is a plain "+"
    done during unsharding).

Numerics: only the routing phase (codes -> top-4 indices) runs in bf16; the
top-4 gate values are recomputed in fp32 from gathered fp32 enc_top rows.
The entire expert pipeline is fp32 with fp32 PSUM accumulation.
"""

import os

import numpy as np
import ml_dtypes

import concourse.bacc as bacc
import concourse.bass as bass
import concourse.mybir as mybir
import concourse.tile as tile
from concourse.bass import IndirectOffsetOnAxis
from concourse.bass_utils import run_bass_kernel_spmd

# ---- problem constants (hardcoded per harness contract) ----
IN_DIM = 2304
SUB = 64
ATOMS = 512
NE = 256
K = 4
P = 128
NCHUNK = IN_DIM // P          # 18 chunks of 128 along input dim
HALF = NCHUNK // 2            # 9 chunks per core-half
ACHUNK = ATOMS // P           # 4 chunks of 128 along atoms
N_CORES = 8

W_COLS = NCHUNK * SUB         # 1152: W_down^T block (chunk-major, m innermost)
E_COLS = ACHUNK * SUB         # 256:  natural E block (atom-chunk-major)
R_COLS = NCHUNK               # 18:   enc_top row (chunk-major)
ET_COLS = ATOMS               # 512:  E^T block (rows 0..63 only, rest zero)
TABA_COLS = W_COLS + E_COLS + R_COLS + ET_COLS   # 1938
ET_OFF = W_COLS + E_COLS + R_COLS                # 1426
R_OFF = W_COLS + E_COLS                          # 1408

ENC_GROUPS = [2, 4, 6, 6]     # enc_top chunks per DMA group (first smallest
NGRP = len(ENC_GROUPS)        # so the PE can start earliest)
N_WARM = int(os.environ.get("KERNEL_WARM_MMS", "30"))
N_PREWARM = int(os.environ.get("KERNEL_PREWARM_MMS", "26"))

OFFSET = float(np.float32(1.0) / np.float32(48.0))  # 1/sqrt(2304), fp32

F32 = mybir.dt.float32
BF16 = mybir.dt.bfloat16
I32 = mybir.dt.int32
U32 = mybir.dt.uint32


def build_program():
    nc = bacc.Bacc("TRN2", target_bir_lowering=False, debug=False,
                   enable_partition_id=False)

    tabW = nc.dram_tensor("tabw", [NE, P, W_COLS], F32,
                          kind="ExternalInput")
    tabB = nc.dram_tensor("tabb", [NE, SUB, ATOMS], F32,
                          kind="ExternalInput")
    tabC = nc.dram_tensor("tabc", [NE, P, E_COLS + R_COLS], F32,
                          kind="ExternalInput")
    ohu32 = nc.dram_tensor("ohu32", [1, 8], U32, kind="ExternalInput")
    encbf = nc.dram_tensor("encbf", [P, NCHUNK, NE], BF16,
                           kind="ExternalInput")
    # fp32 consts: cols 0:18 x (partition-major chunks); col 18 row0-7: unused
    cf32 = nc.dram_tensor("cf32", [P, NCHUNK], F32, kind="ExternalInput")
    # bf16 consts: cols 0:18 x; cols 18:26 row 0: slot one-hot
    cbf16 = nc.dram_tensor("cbf16", [P, NCHUNK + 8], BF16,
                           kind="ExternalInput")
    out_d = nc.dram_tensor("out", [P, HALF], F32, kind="ExternalOutput")

    with tile.TileContext(nc) as tc:
        with (
            tc.tile_pool(name="sb", bufs=1) as sb,
            tc.tile_pool(name="enc", bufs=1) as encp,
            tc.tile_pool(name="ps", bufs=1, space="PSUM") as ps,
        ):
            # ---- phase A: codes = enc_top @ x (bf16, PE) ----
            # first (smallest) enc group + consts on the scalar (ACT) queue,
            # remaining groups on the sync (SP) queue - parallel issue.
            enc_ts = []
            g0 = 0
            for gi, gn in enumerate(ENC_GROUPS):
                enc_t = encp.tile([P, gn, NE], BF16, tag=f"enc{gi}")
                nc.sync.dma_start(enc_t[:], encbf[:, g0:g0 + gn, :])
                enc_ts.append((enc_t, g0, gn))
                g0 += gn
            cb = sb.tile([P, NCHUNK + 8], BF16, tag="cbf")
            nc.scalar.dma_start(cb[:], cbf16[:])
            x_bf = cb[:, 0:NCHUNK]
            x_pm = sb.tile([P, NCHUNK], F32, tag="xpm")
            nc.scalar.dma_start(x_pm[:], cf32[:])
            ohu = sb.tile([1, 8], U32, tag="ohu")
            nc.scalar.dma_start(ohu[:], ohu32[:])

            # on-device constants
            ones_c = sb.tile([P, 1], F32, tag="onesc")
            nc.vector.memset(ones_c[:], 1.0)

            # ---- PE pre-warm: matmuls on a zeroed tile while the first
            # enc-group DMA is in flight, so HAM un-throttles the PE to
            # 2.4 GHz before the codes matmuls start ----
            junk_ps = ps.tile([1, NE], F32, tag="junk")
            if N_PREWARM:
                zwarm = sb.tile([P, P], BF16, tag="zwarm")
                nc.vector.memset(zwarm[:], 0.0)
                for w in range(N_PREWARM):
                    nc.tensor.matmul(
                        junk_ps[:, 0:P],
                        lhsT=zwarm[:, 0:1],
                        rhs=zwarm[:],
                        start=(w == 0),
                        stop=(w == N_PREWARM - 1),
                    )

            codes_ps = ps.tile([1, NE], F32, tag="codes")
            for enc_t, g0, gn in enc_ts:
                for jo in range(gn):
                    jj = g0 + jo
                    nc.tensor.matmul(
                        codes_ps[:],
                        lhsT=x_bf[:, jj:jj + 1],
                        rhs=enc_t[:, jo, :],
                        start=(jj == 0),
                        stop=(jj == NCHUNK - 1),
                    )

            # ---- phase B: top-k (max8 on DVE, reading PSUM) + slot pick ----
            vals = sb.tile([1, 8], F32, tag="vals")
            idxs = sb.tile([1, 8], U32, tag="idxs")
            nc.vector.max_with_indices(vals[:], idxs[:], codes_ps[:])
            scr8 = sb.tile([1, 8], U32, tag="scr8")
            nc.vector.tensor_tensor(
                out=scr8[:], in0=idxs[:], in1=ohu[:],
                op=mybir.AluOpType.mult,
            )
            isel_u = sb.tile([1, 1], U32, tag="iselu")
            with nc.allow_low_precision(
                    reason="one-hot dot on u32 indices; exact"):
                nc.vector.tensor_reduce(
                    out=isel_u[:], in_=scr8[:], axis=mybir.AxisListType.X,
                    op=mybir.AluOpType.add,
                )
            val = nc.values_load(
                isel_u[:],
                engines={mybir.EngineType.SP, mybir.EngineType.Activation},
                min_val=0, max_val=NE - 1, skip_runtime_bounds_check=True,
            )

            # ---- phase C: gather this slot's expert blocks with
            # register-offset direct DMAs (HWDGE).  W first on the SP queue
            # (the s-step long pole); E^T and E-natural+enc_row in parallel
            # on the ACT queue. ----
            gW = sb.tile([P, W_COLS], F32, tag="gw")
            nc.sync.dma_start(gW[:], tabW[bass.ds(val, 1), :, :])
            gB = sb.tile([SUB, ATOMS], F32, tag="gb")
            nc.scalar.dma_start(gB[:], tabB[bass.ds(val, 1), :, :])
            gC = sb.tile([P, E_COLS + R_COLS], F32, tag="gc")
            nc.scalar.dma_start(gC[:], tabC[bass.ds(val, 1), :, :])

            # ---- PE warm-keeper: junk matmuls spanning the topk+gather
            # gap so HAM doesn't re-throttle the PE before the expert
            # pipeline (results written to a scratch PSUM bank, unused) ----
            if N_WARM:
                for w in range(N_WARM):
                    nc.tensor.matmul(
                        junk_ps[:, 0:P],
                        lhsT=zwarm[:, 0:1],
                        rhs=zwarm[:],
                        start=(w == 0),
                        stop=(w == N_WARM - 1),
                    )

            # ---- phase D: expert pipeline (fp32) ----
            # s = W @ x : accumulate over 18 chunks
            s_ps = ps.tile([SUB, 1], F32, tag="s")
            for jj in range(NCHUNK):
                nc.tensor.matmul(
                    s_ps[:],
                    lhsT=gW[:, jj * SUB:(jj + 1) * SUB],
                    rhs=x_pm[:, jj:jj + 1],
                    start=(jj == 0),
                    stop=(jj == NCHUNK - 1),
                )
            s_sb = sb.tile([SUB, 1], F32, tag="ssb")
            nc.vector.tensor_copy(s_sb[:], s_ps[:])

            # c = E @ s : 4 chunks of 128 atoms (lhsT = E^T slabs)
            c_ps = ps.tile([P, ACHUNK], F32, tag="c")
            for ck in range(ACHUNK):
                nc.tensor.matmul(
                    c_ps[:, ck:ck + 1],
                    lhsT=gB[:, ck * P:(ck + 1) * P],
                    rhs=s_sb[:],
                    start=True, stop=True,
                )
            # leaky relu with offset: c >= off ? c : 0.01*c
            cmask = sb.tile([P, ACHUNK], U32, tag="cmask")
            nc.vector.tensor_scalar(
                out=cmask[:], in0=c_ps[:], scalar1=OFFSET, scalar2=None,
                op0=mybir.AluOpType.is_ge,
            )
            cleak = sb.tile([P, ACHUNK], F32, tag="cleak")
            nc.vector.tensor_scalar(
                out=cleak[:], in0=c_ps[:], scalar1=0.01, scalar2=None,
                op0=mybir.AluOpType.mult,
            )
            c_relu = sb.tile([P, ACHUNK], F32, tag="crelu")
            nc.vector.select(c_relu[:], cmask[:], c_ps[:], cleak[:])

            # d^T = c^T @ E : accumulate 4 atom chunks -> [1, 64]
            dT_ps = ps.tile([1, SUB], F32, tag="dt")
            for ck in range(ACHUNK):
                nc.tensor.matmul(
                    dT_ps[:],
                    lhsT=c_relu[:, ck:ck + 1],
                    rhs=gC[:, ck * SUB:(ck + 1) * SUB],
                    start=(ck == 0),
                    stop=(ck == ACHUNK - 1),
                )

            # v = relu_offset(enc_top[i] . x) in fp32 (runs on DVE, parallel
            # with the PE chain above)
            vscr = sb.tile([P, NCHUNK], F32, tag="vscr")
            nc.vector.tensor_tensor(
                out=vscr[:], in0=gC[:, E_COLS:E_COLS + NCHUNK],
                in1=x_pm[:], op=mybir.AluOpType.mult,
            )
            vtmp = sb.tile([P, 1], F32, tag="vtmp")
            nc.vector.tensor_reduce(
                out=vtmp[:], in_=vscr[:], axis=mybir.AxisListType.X,
                op=mybir.AluOpType.add,
            )
            v_ps = ps.tile([1, 1], F32, tag="v")
            nc.tensor.matmul(v_ps[:], lhsT=vtmp[:], rhs=ones_c[:],
                             start=True, stop=True)
            vmask = sb.tile([1, 1], F32, tag="vmask")
            nc.vector.tensor_scalar(
                out=vmask[:], in0=v_ps[:], scalar1=OFFSET, scalar2=None,
                op0=mybir.AluOpType.is_ge,
            )

            # fused [d | v] broadcast to all partitions in one matmul pair
            dtv = sb.tile([1, SUB + 1], F32, tag="dtv")
            nc.vector.tensor_copy(dtv[:, 0:SUB], dT_ps[:])
            nc.vector.tensor_tensor(
                out=dtv[:, SUB:SUB + 1], in0=v_ps[:], in1=vmask[:],
                op=mybir.AluOpType.mult,
            )
            ones_r = sb.tile([1, P], F32, tag="onesr")
            nc.vector.memset(ones_r[:], 1.0)
            bb_ps = ps.tile([P, SUB + 1], F32, tag="bb")
            nc.tensor.matmul(bb_ps[:], lhsT=ones_r[:], rhs=dtv[:],
                             start=True, stop=True)

            # recon half: [128, 9] ; recon[p, jj] = sum_m W^T[p, jj, m]*d[m]
            prod = sb.tile([P, HALF, SUB], F32, tag="prod")
            gA_w3 = gW[:, 0:HALF * SUB].rearrange("p (j m) -> p j m", m=SUB)
            db_bc = bb_ps[:, None, 0:SUB].to_broadcast([P, HALF, SUB])
            nc.vector.tensor_tensor(
                out=prod[:], in0=gA_w3, in1=db_bc, op=mybir.AluOpType.mult,
            )
            recon = sb.tile([P, HALF], F32, tag="recon")
            nc.vector.tensor_reduce(
                out=recon[:], in_=prod[:], axis=mybir.AxisListType.X,
                op=mybir.AluOpType.add,
            )

            # final = recon + v * enc_row[:, :9]
            final = sb.tile([P, HALF], F32, tag="final")
            nc.vector.scalar_tensor_tensor(
                out=final[:],
                in0=gC[:, E_COLS:E_COLS + HALF],
                scalar=bb_ps[:, SUB:SUB + 1],
                in1=recon[:],
                op0=mybir.AluOpType.mult, op1=mybir.AluOpType.add,
            )
            nc.sync.dma_start(out_d[:], final[:])

    nc.compile()
    return nc


def _chunk_order(h):
    """Chunk visit order for core-half h: own half first."""
    own = list(range(h * HALF, (h + 1) * HALF))
    other = list(range((1 - h) * HALF, (2 - h) * HALF))
    return own + other


def _host_prep(x, enc_top, W_down, encoder_weights):
    """Build per-core-half input tables (pure layout transforms)."""
    x = np.asarray(x, np.float32)
    enc_top = np.asarray(enc_top, np.float32)
    W_down = np.asarray(W_down, np.float32)
    E = np.asarray(encoder_weights, np.float32)

    # natural-E block: rows g*128+p, cols ck*64+m = E[g, ck*128+p, m]
    encnat = np.ascontiguousarray(
        E.reshape(NE, ACHUNK, P, SUB).transpose(0, 2, 1, 3)
    ).reshape(NE * P, E_COLS)
    # E^T table: [g, s, a] = E[g, a, s]
    tabB = np.ascontiguousarray(E.transpose(0, 2, 1))

    Wr = W_down.reshape(NE, SUB, NCHUNK, P)          # [g, m, j, p]
    Er = enc_top.reshape(NE, NCHUNK, P)              # [g, j, p]

    per_half = {}
    for h in (0, 1):
        order = _chunk_order(h)
        tabW = np.ascontiguousarray(
            Wr[:, :, order, :].transpose(0, 3, 2, 1)  # [g, p, jj, m]
        ).reshape(NE, P, W_COLS)
        encrow = (
            Er[:, order, :].transpose(0, 2, 1)        # [g, p, jj]
        ).reshape(NE * P, R_COLS)
        tabC = np.concatenate([encnat, encrow], axis=1).reshape(
            NE, P, E_COLS + R_COLS)

        x_pm = np.ascontiguousarray(
            x.reshape(NCHUNK, P)[order, :].T)          # [p, jj]
        encbf = np.ascontiguousarray(
            Er[:, order, :].transpose(2, 1, 0)         # [p, jj, g]
        ).astype(ml_dtypes.bfloat16)
        per_half[h] = dict(
            tabw=tabW,
            tabc=tabC,
            cf32=x_pm,
            xbf=x_pm.astype(ml_dtypes.bfloat16),
            encbf=encbf,
        )

    in_maps = []
    for c in range(N_CORES):
        h, slot = c // 4, c % 4
        ph = per_half[h]
        cbf = np.zeros((P, NCHUNK + 8), ml_dtypes.bfloat16)
        cbf[:, :NCHUNK] = ph["xbf"]
        ohu = np.zeros((1, 8), np.uint32)
        ohu[0, slot] = 1
        in_maps.append({
            "tabw": ph["tabw"],
            "tabb": tabB,
            "tabc": ph["tabc"],
            "encbf": ph["encbf"],
            "cf32": ph["cf32"],
            "cbf16": cbf,
            "ohu32": ohu,
        })
    return in_maps


def _assemble(results):
    out = np.zeros(IN_DIM, np.float32).reshape(NCHUNK, P)
    for c in range(N_CORES):
        h = c // 4
        own = _chunk_order(h)[:HALF]
        out[own, :] += results[c]["out"].T
    return out.reshape(IN_DIM)


_NC_CACHE = {}
LAST_RESULT = {}


def kernel(x, enc_top, W_down, encoder_weights):
    in_maps = _host_prep(x, enc_top, W_down, encoder_weights)
    if "nc" not in _NC_CACHE:
        _NC_CACHE["nc"] = build_program()
    nc = _NC_CACHE["nc"]

    if os.environ.get("BASS_SIM") == "1":
        from concourse.bass_interp import CoreSim
        sim_cores = os.environ.get("BASS_SIM_CORES")
        cores = (
            [int(t) for t in sim_cores.split(",")] if sim_cores
            else range(N_CORES)
        )
        results = [None] * N_CORES
        for c in cores:
            nc_c = build_program()
            sim = CoreSim(nc_c)
            for name, arr in in_maps[c].items():
                sim.tensor(name)[:] = arr
            sim.simulate()
            results[c] = {"out": np.array(sim.tensor("out"))}
        for c in range(N_CORES):
            if results[c] is None:
                results[c] = {"out": np.zeros((P, HALF), np.float32)}
        return _assemble(results)

    trace = os.environ.get("BASS_TRACE") == "1"
    if trace:
        _ensure_trace_hook()
    res = run_bass_kernel_spmd(
        nc, in_maps, core_ids=list(range(N_CORES)),
        trace=trace,
    )
    LAST_RESULT["res"] = res
    return _assemble(res.results)


def _ensure_trace_hook():
    """Install the axon NTFF profile hook if antenv.axon_hooks is absent."""
    try:
        from antenv.axon_hooks import get_axon_ntff_profile_hook  # noqa
        return
    except ImportError:
        pass
    import sys
    import types
    try:
        from trn_agent_boot.trn_boot import _ntff_profile_via_ctypes
    except ImportError:
        return
    hook = _ntff_profile_via_ctypes("/opt/axon/libaxon_pjrt.so")
    mod = types.ModuleType("antenv.axon_hooks")
    mod._hook = hook
    mod.get_axon_ntff_profile_hook = lambda: mod._hook
    mod.set_axon_ntff_profile_hook = lambda h: setattr(mod, "_hook", h)
    import antenv
    sys.modules["antenv.axon_hooks"] = mod
    antenv.axon_hooks = mod


if __name__ == "__main__":
    nc = build_program()
    print("program built ok")



# revision 15
# speedup vs baseline: 1.0980x; 1.0980x over previous
"""Trainium2 Bass kernel for single-token MoE routing (nn_MixtureOfExperts_v2).

Problem:
    x [2304]; enc_top [256, 2304]; W_down [256, 64, 2304]; encoder_weights
    [256, 512, 64].
    codes = relu_offset(enc_top @ x)           (slope 0.0, offset 1/48)
    top4 values/indices of codes
    per selected expert i (gate v):
        s = W_down[i] @ x                      [64]
        c = relu_offset(E[i] @ s, slope 0.01)  [512]
        d = E[i]^T @ c                         [64]
        recon += W_down[i]^T @ d               [2304]
        recon += v * enc_top[i]
    output = recon                             [2304]

Distribution (8 cores, no collectives):
    Every core loads a replicated fp8 transposed copy of enc_top, computes
    all 256 codes on the PE, and runs top-4 on the vector engine
    (max_with_indices), so all cores agree on the routing.  Core c then
    processes selected slot (c % 4) alone: it gathers that expert's weights
    (bf16) with two register-offset direct DMAs and runs the expert
    pipeline.  Cores c and c+4 process the same slot but emit complementary
    halves of the 2304-dim reconstruction (the per-core tables are built
    with the core's half of the input-dim chunks first, so the program is
    identical across cores - pure SPMD with per-core constants).  The host
    sums the 8 partial outputs (the cross-core reduction is a plain "+"
    done during unsharding).

Expert pipeline dataflow (v2): the skinny matvecs (s = W @ x and
d = E^T @ c) run on the vector engine as broadcast-multiply + reduce over
the free dim, leaving only cross-partition sums / broadcasts to the PE
(two matmuls against a constant all-ones weight).  This avoids the
~125ns/matmul LDWEIGHTS floor of a PE-side chunk loop and is insensitive
to the HAM clock throttle.  All gathered tables are bf16; routing runs in
fp8 (selection-only; the gate value is recomputed from bf16 tables).
"""

import os

import numpy as np
import ml_dtypes

import concourse.bacc as bacc
import concourse.bass as bass
import concourse.mybir as mybir
import concourse.tile as tile
from concourse.bass_utils import run_bass_kernel_spmd

# ---- problem constants (hardcoded per harness contract) ----
IN_DIM = 2304
SUB = 64
ATOMS = 512
NE = 256
K = 4
P = 128
NCHUNK = IN_DIM // P          # 18 chunks of 128 along input dim
HALF = NCHUNK // 2            # 9 chunks per core-half
ACHUNK = ATOMS // P           # 4 chunks of 128 along atoms
N_CORES = 8

W_COLS = SUB * NCHUNK         # 1152: W^T block, m-major (jj innermost)
WR_COLS = HALF * SUB          # 576:  W^T own-half block, jj-major (m inner)
E_COLS = ACHUNK * SUB         # 256:  E natural block, ck-major (m inner)
MC_OFF = E_COLS               # 256:  E natural block, m-major (ck inner)
R_OFF = 2 * E_COLS            # 512:  enc_top row (chunk-major)
R_COLS = NCHUNK               # 18
TABE_COLS = R_OFF + R_COLS    # 530
RA = 5                        # recon first-half chunks (second: HALF-RA)

# enc chunk groups per DMA: (queue, nchunks); first smallest so the PE can
# start the codes matmuls earliest.  sync and scalar queues stream
# concurrently.
ENC_GROUPS = [("sync", 2), ("sync", 6), ("scalar", 6), ("scalar", 4)]
N_PREWARM = int(os.environ.get("KERNEL_PREWARM_MMS", "14"))

OFFSET = float(np.float32(1.0) / np.float32(48.0))  # 1/sqrt(2304), fp32

F32 = mybir.dt.float32
BF16 = mybir.dt.bfloat16
F8 = mybir.dt.float8e4
I32 = mybir.dt.int32
U32 = mybir.dt.uint32


def build_program():
    nc = bacc.Bacc("TRN2", target_bir_lowering=False, debug=False,
                   enable_partition_id=False)

    tabW = nc.dram_tensor("tabw", [NE, P, W_COLS], BF16,
                          kind="ExternalInput")
    tabR = nc.dram_tensor("tabr", [NE, P, WR_COLS], BF16,
                          kind="ExternalInput")
    tabE = nc.dram_tensor("tabe", [NE, P, TABE_COLS], BF16,
                          kind="ExternalInput")
    encq = nc.dram_tensor("encq", [P, NCHUNK, NE], F8, kind="ExternalInput")
    # packed consts, bf16-typed blob:
    #   cols 0:18    x bf16 (chunk-major, core-half order)
    #   cols 18:27   x fp8 (bitcast view, 18 values)
    #   cols 28:44   slot one-hot uint32 (bitcast view, 8 values, all rows)
    cblob = nc.dram_tensor("cblob", [P, 44], BF16, kind="ExternalInput")
    outA = nc.dram_tensor("outa", [P, RA], F32, kind="ExternalOutput")
    outB = nc.dram_tensor("outb", [P, HALF - RA], F32,
                          kind="ExternalOutput")

    with tile.TileContext(nc) as tc:
        with (
            tc.tile_pool(name="sb", bufs=1) as sb,
            tc.tile_pool(name="enc", bufs=1) as encp,
            tc.tile_pool(name="ps", bufs=1, space="PSUM") as ps,
        ):
            # ---- phase A: codes = enc_top @ x (fp8, PE) ----
            # consts FIRST on the scalar queue - the codes matmuls need the
            # x weights before any enc group
            cb = sb.tile([P, 44], BF16, tag="cblob")
            nc.scalar.dma_start(cb[:], cblob[:])
            x_bf = cb[:, 0:NCHUNK]
            x_q8 = cb[:, 18:27].bitcast(F8)          # [P, 18]
            ohu = cb[0:1, 28:44].bitcast(U32)        # [1, 8]
            enc_ts = []
            g0 = 0
            for gi, (q, gn) in enumerate(ENC_GROUPS):
                enc_t = encp.tile([P, gn, NE], F8, tag=f"enc{gi}")
                eng = nc.sync if q == "sync" else nc.scalar
                eng.dma_start(enc_t[:], encq[:, g0:g0 + gn, :])
                enc_ts.append((enc_t, g0, gn))
                g0 += gn

            # on-device constants
            ones_bf = sb.tile([P, P], BF16, tag="onesbf")
            nc.vector.memset(ones_bf[:], 1.0)

            # ---- PE pre-warm: matmuls on the ones tile while the first
            # enc-group DMA is in flight, so HAM un-throttles the PE to
            # 2.4 GHz before the codes matmuls start ----
            junk_ps = ps.tile([1, NE], F32, tag="junk")
            if N_PREWARM:
                for w in range(N_PREWARM):
                    nc.tensor.matmul(
                        junk_ps[:, 0:P],
                        lhsT=ones_bf[:, 0:1],
                        rhs=ones_bf[:],
                        start=(w == 0),
                        stop=(w == N_PREWARM - 1),
                    )

            codes_ps = ps.tile([1, NE], F32, tag="codes")
            for enc_t, g0, gn in enc_ts:
                for jo in range(gn):
                    jj = g0 + jo
                    nc.tensor.matmul(
                        codes_ps[:],
                        lhsT=x_q8[:, jj:jj + 1],
                        rhs=enc_t[:, jo, :],
                        start=(jj == 0),
                        stop=(jj == NCHUNK - 1),
                    )

            # ---- phase B: top-k (max8 on DVE, reading PSUM) + slot pick ----
            vals = sb.tile([1, 8], F32, tag="vals")
            idxs = sb.tile([1, 8], U32, tag="idxs")
            nc.vector.max_with_indices(vals[:], idxs[:], codes_ps[:])
            scr8 = sb.tile([1, 8], U32, tag="scr8")
            nc.vector.tensor_tensor(
                out=scr8[:], in0=idxs[:], in1=ohu,
                op=mybir.AluOpType.mult,
            )
            isel_u = sb.tile([1, 1], U32, tag="iselu")
            with nc.allow_low_precision(
                    reason="one-hot dot on u32 indices; exact"):
                nc.vector.tensor_reduce(
                    out=isel_u[:], in_=scr8[:], axis=mybir.AxisListType.X,
                    op=mybir.AluOpType.add,
                )
            val = nc.values_load(
                isel_u[:],
                engines={mybir.EngineType.SP, mybir.EngineType.Activation},
                min_val=0, max_val=NE - 1, skip_runtime_bounds_check=True,
            )

            # ---- phase C: gather this slot's expert blocks with
            # register-offset direct DMAs (HWDGE).  W (m-major, for s) and
            # the recon block (jj-major own half) are separate DMAs so the
            # s partials can start before the recon block lands. ----
            gW = sb.tile([P, W_COLS], BF16, tag="gw")
            nc.sync.dma_start(gW[:], tabW[bass.ds(val, 1), :, :])
            gE = sb.tile([P, TABE_COLS], BF16, tag="ge")
            nc.scalar.dma_start(gE[:], tabE[bass.ds(val, 1), :, :])
            gR = sb.tile([P, WR_COLS], BF16, tag="gr")
            nc.sync.dma_start(gR[:], tabR[bass.ds(val, 1), :, :])

            # ---- phase D: expert pipeline (bf16 DVE/PE hybrid) ----
            # per-partition partials of d and of the gate dot, summed and
            # broadcast by one ones-weight matmul: bb = ones^T @ [d | v]
            p2 = sb.tile([P, SUB + 1], BF16, tag="p2")

            # gate partials: enc_row * x, reduced over chunks (runs as soon
            # as gE lands; off the s critical path)
            vprod = sb.tile([P, NCHUNK], BF16, tag="vprod")
            nc.vector.tensor_tensor(
                out=vprod[:], in0=gE[:, R_OFF:R_OFF + NCHUNK], in1=x_bf,
                op=mybir.AluOpType.mult,
            )
            with nc.allow_low_precision(reason="bf16 partials, fp32 accum"):
                nc.vector.tensor_reduce(
                    out=p2[:, SUB:SUB + 1], in_=vprod[:],
                    axis=mybir.AxisListType.X, op=mybir.AluOpType.add,
                )

            # s partials: W^T (m-major) * x, reduced over chunks -> [P, 64]
            gW_mj = gW[:].rearrange("p (m j) -> p m j", j=NCHUNK)
            sprod = sb.tile([P, SUB, NCHUNK], BF16, tag="sprod")
            nc.vector.tensor_tensor(
                out=sprod[:], in0=gW_mj,
                in1=x_bf[:, None, :].to_broadcast([P, SUB, NCHUNK]),
                op=mybir.AluOpType.mult,
            )
            spart = sb.tile([P, SUB], BF16, tag="spart")
            with nc.allow_low_precision(reason="bf16 partials, fp32 accum"):
                nc.vector.tensor_reduce(
                    out=spart[:], in_=sprod[:], axis=mybir.AxisListType.X,
                    op=mybir.AluOpType.add,
                )

            # s broadcast to all partitions: sb_ps = ones^T @ spart
            sb_ps = ps.tile([P, SUB], F32, tag="sbps")
            nc.tensor.matmul(sb_ps[:], lhsT=ones_bf[:], rhs=spart[:],
                             start=True, stop=True)

            # c = E @ s: E natural [p, ck, m] * s broadcast (read straight
            # from PSUM), reduce over m
            gE_cm = gE[:, 0:E_COLS].rearrange("p (c m) -> p c m", m=SUB)
            cprod = sb.tile([P, ACHUNK, SUB], BF16, tag="cprod")
            nc.vector.tensor_tensor(
                out=cprod[:], in0=gE_cm,
                in1=sb_ps[:, None, :].to_broadcast([P, ACHUNK, SUB]),
                op=mybir.AluOpType.mult,
            )
            c_sb = sb.tile([P, ACHUNK], F32, tag="csb")
            nc.vector.tensor_reduce(
                out=c_sb[:], in_=cprod[:], axis=mybir.AxisListType.X,
                op=mybir.AluOpType.add,
            )

            # leaky relu with offset: c * (0.01 + 0.99*(c >= off))
            cmask = sb.tile([P, ACHUNK], F32, tag="cmask")
            nc.vector.tensor_scalar(
                out=cmask[:], in0=c_sb[:], scalar1=OFFSET, scalar2=None,
                op0=mybir.AluOpType.is_ge,
            )
            cfac = sb.tile([P, ACHUNK], F32, tag="cfac")
            nc.vector.tensor_scalar(
                out=cfac[:], in0=cmask[:], scalar1=0.99, scalar2=0.01,
                op0=mybir.AluOpType.mult, op1=mybir.AluOpType.add,
            )
            c_relu = sb.tile([P, ACHUNK], BF16, tag="crelu")
            nc.vector.tensor_tensor(
                out=c_relu[:], in0=c_sb[:], in1=cfac[:],
                op=mybir.AluOpType.mult,
            )

            # d partials: E natural m-major block [p, m, ck] * c, reduce
            # over ck (contiguous in0)
            gE_mc = gE[:, MC_OFF:MC_OFF + E_COLS].rearrange(
                "p (m c) -> p m c", c=ACHUNK)
            dprod = sb.tile([P, SUB, ACHUNK], BF16, tag="dprod")
            nc.vector.tensor_tensor(
                out=dprod[:], in0=gE_mc,
                in1=c_relu[:, None, :].to_broadcast([P, SUB, ACHUNK]),
                op=mybir.AluOpType.mult,
            )
            with nc.allow_low_precision(reason="bf16 partials, fp32 accum"):
                nc.vector.tensor_reduce(
                    out=p2[:, 0:SUB], in_=dprod[:],
                    axis=mybir.AxisListType.X, op=mybir.AluOpType.add,
                )

            # broadcast [d | v]: bb = ones^T @ p2  (col 64 sums the gate
            # partials at the same time)
            bb_ps = ps.tile([P, SUB + 1], F32, tag="bb")
            nc.tensor.matmul(bb_ps[:], lhsT=ones_bf[:], rhs=p2[:],
                             start=True, stop=True)

            # top-level gate: v = v_raw * (v_raw >= off)  (slope 0.0)
            gmask = sb.tile([P, 1], F32, tag="gmask")
            nc.vector.tensor_scalar(
                out=gmask[:], in0=bb_ps[:, SUB:SUB + 1], scalar1=OFFSET,
                scalar2=None, op0=mybir.AluOpType.is_ge,
            )
            gv = sb.tile([P, 1], F32, tag="gv")
            nc.vector.tensor_tensor(
                out=gv[:], in0=bb_ps[:, SUB:SUB + 1], in1=gmask[:],
                op=mybir.AluOpType.mult,
            )

            # recon, in two pieces so the first output DMA overlaps the
            # second piece's vector work: W^T own-half jj-major (contiguous)
            # * d broadcast (read straight from PSUM), reduce over m
            gR_jm = gR[:].rearrange("p (j m) -> p j m", m=SUB)
            for pi, (j0, j1, eng, od) in enumerate(
                    [(0, RA, nc.sync, outA),
                     (RA, HALF, nc.scalar, outB)]):
                jn = j1 - j0
                rprod = sb.tile([P, jn, SUB], BF16, tag=f"rprod{pi}")
                nc.vector.tensor_tensor(
                    out=rprod[:], in0=gR_jm[:, j0:j1, :],
                    in1=bb_ps[:, None, 0:SUB].to_broadcast([P, jn, SUB]),
                    op=mybir.AluOpType.mult,
                )
                recon = sb.tile([P, jn], F32, tag=f"recon{pi}")
                nc.vector.tensor_reduce(
                    out=recon[:], in_=rprod[:], axis=mybir.AxisListType.X,
                    op=mybir.AluOpType.add,
                )
                final = sb.tile([P, jn], F32, tag=f"final{pi}")
                nc.vector.scalar_tensor_tensor(
                    out=final[:],
                    in0=gE[:, R_OFF + j0:R_OFF + j1],
                    scalar=gv[:],
                    in1=recon[:],
                    op0=mybir.AluOpType.mult, op1=mybir.AluOpType.add,
                )
                eng.dma_start(od[:], final[:])

    nc.compile()
    return nc


def _chunk_order(h):
    """Chunk visit order for core-half h: own half first."""
    own = list(range(h * HALF, (h + 1) * HALF))
    other = list(range((1 - h) * HALF, (2 - h) * HALF))
    return own + other


def _host_prep(x, enc_top, W_down, encoder_weights):
    """Build per-core-half input tables (pure layout transforms)."""
    x = np.asarray(x, np.float32)
    enc_top = np.asarray(enc_top, np.float32)
    W_down = np.asarray(W_down, np.float32)
    E = np.asarray(encoder_weights, np.float32)

    # E natural blocks: ck-major [g, p, ck*64+m] and m-major
    # [g, p, m*4+ck], both = E[g, ck*128+p, m]
    Enat = E.reshape(NE, ACHUNK, P, SUB)
    encnat_cm = np.ascontiguousarray(
        Enat.transpose(0, 2, 1, 3)
    ).reshape(NE, P, E_COLS).astype(ml_dtypes.bfloat16)
    encnat_mc = np.ascontiguousarray(
        Enat.transpose(0, 2, 3, 1)
    ).reshape(NE, P, E_COLS).astype(ml_dtypes.bfloat16)

    Wr = W_down.reshape(NE, SUB, NCHUNK, P)          # [g, m, j, p]
    Er = enc_top.reshape(NE, NCHUNK, P)              # [g, j, p]

    per_half = {}
    for h in (0, 1):
        order = _chunk_order(h)
        # W^T m-major: [g, p, m*18+jj] = W[g, m, order[jj]*128+p]
        tabW = np.ascontiguousarray(
            Wr[:, :, order, :].transpose(0, 3, 1, 2)  # [g, p, m, jj]
        ).reshape(NE, P, W_COLS).astype(ml_dtypes.bfloat16)
        # W^T own-half jj-major: [g, p, jj*64+m]
        tabR = np.ascontiguousarray(
            Wr[:, :, order[:HALF], :].transpose(0, 3, 2, 1)  # [g, p, j, m]
        ).reshape(NE, P, WR_COLS).astype(ml_dtypes.bfloat16)
        encrow = (
            Er[:, order, :].transpose(0, 2, 1)        # [g, p, jj]
        ).astype(ml_dtypes.bfloat16)
        tabE = np.concatenate([encnat_cm, encnat_mc, encrow], axis=2)

        x_pm = np.ascontiguousarray(
            x.reshape(NCHUNK, P)[order, :].T)          # [p, jj]
        encq = np.ascontiguousarray(
            Er[:, order, :].transpose(2, 1, 0)         # [p, jj, g]
        ).astype(ml_dtypes.float8_e4m3)
        per_half[h] = dict(
            tabw=tabW,
            tabr=tabR,
            tabe=tabE,
            xbf=x_pm.astype(ml_dtypes.bfloat16),
            xq8=x_pm.astype(ml_dtypes.float8_e4m3),
            encq=encq,
        )

    in_maps = []
    for c in range(N_CORES):
        h, slot = c // 4, c % 4
        ph = per_half[h]
        blob = np.zeros((P, 88), np.uint8)
        blob[:, 0:36] = ph["xbf"].view(np.uint8)
        blob[:, 36:54] = ph["xq8"].view(np.uint8)
        ohu = np.zeros(8, np.uint32)
        ohu[slot] = 1
        blob[:, 56:88] = ohu.view(np.uint8)[None, :]
        in_maps.append({
            "tabw": ph["tabw"],
            "tabr": ph["tabr"],
            "tabe": ph["tabe"],
            "encq": ph["encq"],
            "cblob": blob.view(ml_dtypes.bfloat16),
        })
    return in_maps


def _assemble(results):
    out = np.zeros(IN_DIM, np.float32).reshape(NCHUNK, P)
    for c in range(N_CORES):
        h = c // 4
        own = _chunk_order(h)[:HALF]
        full = np.concatenate([results[c]["outa"], results[c]["outb"]],
                              axis=1)
        out[own, :] += full.T
    return out.reshape(IN_DIM)


_NC_CACHE = {}
LAST_RESULT = {}


def kernel(x, enc_top, W_down, encoder_weights):
    in_maps = _host_prep(x, enc_top, W_down, encoder_weights)
    if "nc" not in _NC_CACHE:
        _NC_CACHE["nc"] = build_program()
    nc = _NC_CACHE["nc"]

    if os.environ.get("BASS_SIM") == "1":
        from concourse.bass_interp import CoreSim
        sim_cores = os.environ.get("BASS_SIM_CORES")
        cores = (
            [int(t) for t in sim_cores.split(",")] if sim_cores
            else range(N_CORES)
        )
        results = [None] * N_CORES
        for c in cores:
            nc_c = build_program()
            sim = CoreSim(nc_c)
            for name, arr in in_maps[c].items():
                sim.tensor(name)[:] = arr
            sim.simulate()
            results[c] = {"outa": np.array(sim.tensor("outa")),
                          "outb": np.array(sim.tensor("outb"))}
        for c in range(N_CORES):
            if results[c] is None:
                results[c] = {"outa": np.zeros((P, RA), np.float32),
                              "outb": np.zeros((P, HALF - RA), np.float32)}
        return _assemble(results)

    trace = os.environ.get("BASS_TRACE") == "1"
    if trace:
        _ensure_trace_hook()
    res = run_bass_kernel_spmd(
        nc, in_maps, core_ids=list(range(N_CORES)),
        trace=trace,
    )
    LAST_RESULT["res"] = res
    return _assemble(res.results)


def _ensure_trace_hook():
    """Install the axon NTFF profile hook if antenv.axon_hooks is absent."""
    try:
        from antenv.axon_hooks import get_axon_ntff_profile_hook  # noqa
        return
    except ImportError:
        pass
    import sys
    import types
    try:
        from trn_agent_boot.trn_boot import _ntff_profile_via_ctypes
    except ImportError:
        return
    hook = _ntff_profile_via_ctypes("/opt/axon/libaxon_pjrt.so")
    mod = types.ModuleType("antenv.axon_hooks")
    mod._hook = hook
    mod.get_axon_ntff_profile_hook = lambda: mod._hook
    mod.set_axon_ntff_profile_hook = lambda h: setattr(mod, "_hook", h)
    import antenv
    sys.modules["antenv.axon_hooks"] = mod
    antenv.axon_hooks = mod


if __name__ == "__main__":
    nc = build_program()
    print("program built ok")


# revision 24
# speedup vs baseline: 1.2215x; 1.1124x over previous
"""Trainium2 Bass kernel for single-token MoE routing (nn_MixtureOfExperts_v2).

Problem:
    x [2304]; enc_top [256, 2304]; W_down [256, 64, 2304]; encoder_weights
    [256, 512, 64].
    codes = relu_offset(enc_top @ x)           (slope 0.0, offset 1/48)
    top4 values/indices of codes
    per selected expert i (gate v):
        s = W_down[i] @ x                      [64]
        c = relu_offset(E[i] @ s, slope 0.01)  [512]
        d = E[i]^T @ c                         [64]
        recon += W_down[i]^T @ d               [2304]
        recon += v * enc_top[i]
    output = recon                             [2304]

Distribution (8 cores, no collectives):
    Every core loads a replicated fp8 transposed copy of enc_top, computes
    all 256 codes on the PE, and runs top-4 on the vector engine
    (max_with_indices), so all cores agree on the routing.  Core c then
    processes selected slot (c % 4) alone: it gathers that expert's weights
    (bf16) with two register-offset direct DMAs and runs the expert
    pipeline.  Cores c and c+4 process the same slot but emit complementary
    halves of the 2304-dim reconstruction (the per-core tables are built
    with the core's half of the input-dim chunks first, so the program is
    identical across cores - pure SPMD with per-core constants).  The host
    sums the 8 partial outputs (the cross-core reduction is a plain "+"
    done during unsharding).

Expert pipeline dataflow (v2): the skinny matvecs (s = W @ x and
d = E^T @ c) run on the vector engine as broadcast-multiply + reduce over
the free dim, leaving only cross-partition sums / broadcasts to the PE
(two matmuls against a constant all-ones weight).  This avoids the
~125ns/matmul LDWEIGHTS floor of a PE-side chunk loop and is insensitive
to the HAM clock throttle.  All gathered tables are bf16; routing runs in
fp8 (selection-only; the gate value is recomputed from bf16 tables).
"""

import os

import numpy as np
import ml_dtypes

import concourse.bacc as bacc
import concourse.bass as bass
import concourse.mybir as mybir
import concourse.tile as tile
from concourse.bass_utils import run_bass_kernel_spmd

# ---- problem constants (hardcoded per harness contract) ----
IN_DIM = 2304
SUB = 64
ATOMS = 512
NE = 256
K = 4
P = 128
NCHUNK = IN_DIM // P          # 18 chunks of 128 along input dim
HALF = NCHUNK // 2            # 9 chunks per core-half
ACHUNK = ATOMS // P           # 4 chunks of 128 along atoms
N_CORES = 8

W_COLS = SUB * NCHUNK         # 1152: W^T block, m-major (jj innermost)
WR_COLS = HALF * SUB          # 576:  W^T own-half block, jj-major (m inner)
E_COLS = ACHUNK * SUB         # 256:  E natural block, ck-major (m inner)
MC_OFF = E_COLS               # 256:  E natural block, m-major (ck inner)
R_OFF = 2 * E_COLS            # 512:  enc_top row (chunk-major)
R_COLS = NCHUNK               # 18
TABE_COLS = R_OFF + R_COLS    # 530
RA = 5                        # recon first-half chunks (second: HALF-RA)

# enc chunk groups per DMA after the merged first group: (queue, nchunks).
# sync and scalar queues stream concurrently.
G0_CHUNKS = 2                 # chunks merged with the consts in encg0
ENC_GROUPS = [("sync", 6), ("scalar", 6), ("scalar", 4)]
G0_BYTES = G0_CHUNKS * NE     # 512
XBF_OFF = G0_BYTES            # 512: x bf16 (36 bytes)
XQ8_OFF = XBF_OFF + 36        # 548: x fp8 (18 bytes)
OHU_OFF = 568                 # one-hot u32 (32 bytes), 4-aligned
G0_COLS = 600
N_PREWARM = int(os.environ.get("KERNEL_PREWARM_MMS", "22"))

OFFSET = float(np.float32(1.0) / np.float32(48.0))  # 1/sqrt(2304), fp32

F32 = mybir.dt.float32
BF16 = mybir.dt.bfloat16
F8 = mybir.dt.float8e4
I32 = mybir.dt.int32
U32 = mybir.dt.uint32


def build_program():
    nc = bacc.Bacc("TRN2", target_bir_lowering=False, debug=False,
                   enable_partition_id=False)

    tabW = nc.dram_tensor("tabw", [NE, P, W_COLS], BF16,
                          kind="ExternalInput")
    tabR = nc.dram_tensor("tabr", [NE, P, WR_COLS], BF16,
                          kind="ExternalInput")
    tabE = nc.dram_tensor("tabe", [NE, P, TABE_COLS], BF16,
                          kind="ExternalInput")
    # merged first group: enc chunks 0:2 (fp8) + x bf16 + x fp8 + one-hot,
    # one DMA -> one semaphore gating the first codes matmuls
    encg0 = nc.dram_tensor("encg0", [P, G0_COLS], mybir.dt.uint8,
                           kind="ExternalInput")
    encq = nc.dram_tensor("encq", [P, NCHUNK - G0_CHUNKS, NE], F8,
                          kind="ExternalInput")
    out_d = nc.dram_tensor("out", [P, HALF], F32, kind="ExternalOutput")

    with tile.TileContext(nc) as tc:
        with (
            tc.tile_pool(name="sb", bufs=1) as sb,
            tc.tile_pool(name="enc", bufs=1) as encp,
            tc.tile_pool(name="ps", bufs=1, space="PSUM") as ps,
        ):
            # ---- phase A: codes = enc_top @ x (fp8, PE) ----
            g0t = sb.tile([P, G0_COLS], mybir.dt.uint8, tag="encg0")
            nc.sync.dma_start(g0t[:], encg0[:])
            enc0 = g0t[:, 0:G0_BYTES].bitcast(F8).rearrange(
                "p (g e) -> p g e", e=NE)
            x_bf = g0t[:, XBF_OFF:XBF_OFF + 36].bitcast(BF16)   # [P, 18]
            x_q8 = g0t[:, XQ8_OFF:XQ8_OFF + 18].bitcast(F8)     # [P, 18]
            ohu = g0t[0:1, OHU_OFF:OHU_OFF + 32].bitcast(U32)   # [1, 8]
            enc_ts = [(enc0, 0, G0_CHUNKS)]
            g0 = G0_CHUNKS
            for gi, (q, gn) in enumerate(ENC_GROUPS):
                enc_t = encp.tile([P, gn, NE], F8, tag=f"enc{gi}")
                eng = nc.sync if q == "sync" else nc.scalar
                eng.dma_start(enc_t[:], encq[:, g0 - G0_CHUNKS:
                                             g0 - G0_CHUNKS + gn, :])
                enc_ts.append((enc_t, g0, gn))
                g0 += gn

            # on-device constants
            ones_bf = sb.tile([P, P], BF16, tag="onesbf")
            nc.vector.memset(ones_bf[:], 1.0)

            # ---- PE pre-warm: matmuls on the ones tile while the first
            # enc-group DMA is in flight, so HAM un-throttles the PE to
            # 2.4 GHz before the codes matmuls start ----
            junk_ps = ps.tile([1, NE], F32, tag="junk")
            if N_PREWARM:
                for w in range(N_PREWARM):
                    nc.tensor.matmul(
                        junk_ps[:, 0:P],
                        lhsT=ones_bf[:, 0:1],
                        rhs=ones_bf[:],
                        start=(w == 0),
                        stop=(w == N_PREWARM - 1),
                    )

            codes_ps = ps.tile([1, NE], F32, tag="codes")
            for enc_t, g0, gn in enc_ts:
                for jo in range(gn):
                    jj = g0 + jo
                    nc.tensor.matmul(
                        codes_ps[:],
                        lhsT=x_q8[:, jj:jj + 1],
                        rhs=enc_t[:, jo, :],
                        start=(jj == 0),
                        stop=(jj == NCHUNK - 1),
                    )

            # ---- phase B: top-k (max8 on DVE, reading PSUM) + slot pick ----
            vals = sb.tile([1, 8], F32, tag="vals")
            idxs = sb.tile([1, 8], U32, tag="idxs")
            nc.vector.max_with_indices(vals[:], idxs[:], codes_ps[:])
            scr8 = sb.tile([1, 8], U32, tag="scr8")
            nc.vector.tensor_tensor(
                out=scr8[:], in0=idxs[:], in1=ohu,
                op=mybir.AluOpType.mult,
            )
            isel_u = sb.tile([1, 1], U32, tag="iselu")
            with nc.allow_low_precision(
                    reason="one-hot dot on u32 indices; exact"):
                nc.vector.tensor_reduce(
                    out=isel_u[:], in_=scr8[:], axis=mybir.AxisListType.X,
                    op=mybir.AluOpType.add,
                )
            val = nc.values_load(
                isel_u[:],
                engines={mybir.EngineType.SP, mybir.EngineType.Activation},
                min_val=0, max_val=NE - 1, skip_runtime_bounds_check=True,
            )

            # ---- phase C: gather this slot's expert blocks with
            # register-offset direct DMAs (HWDGE).  W (m-major, for s) and
            # the recon block (jj-major own half) are separate DMAs so the
            # s partials can start before the recon block lands. ----
            gW = sb.tile([P, W_COLS], BF16, tag="gw")
            nc.sync.dma_start(gW[:], tabW[bass.ds(val, 1), :, :])
            gE = sb.tile([P, TABE_COLS], BF16, tag="ge")
            nc.scalar.dma_start(gE[:], tabE[bass.ds(val, 1), :, :])
            gR = sb.tile([P, WR_COLS], BF16, tag="gr")
            nc.sync.dma_start(gR[:], tabR[bass.ds(val, 1), :, :])

            # ---- phase D: expert pipeline (bf16 DVE/PE hybrid) ----
            # per-partition partials of d and of the gate dot, summed and
            # broadcast by one ones-weight matmul: bb = ones^T @ [d | v]
            p2 = sb.tile([P, SUB + 1], BF16, tag="p2")

            # gate dot: v_raw = sum(enc_row * x).  Runs entirely on the
            # otherwise-idle gpsimd engine (its reduce sums across
            # partitions too), so it never blocks the DVE chain.  v_raw
            # lands in p2[0, 64] with the rest of that column zeroed; the
            # bb matmul's column sum then broadcasts it to all partitions.
            nc.gpsimd.memset(p2[:, SUB:SUB + 1], 0.0)
            vprod = sb.tile([P, NCHUNK], BF16, tag="vprod")
            nc.gpsimd.tensor_tensor(
                out=vprod[:], in0=gE[:, R_OFF:R_OFF + NCHUNK], in1=x_bf,
                op=mybir.AluOpType.mult,
            )
            with nc.allow_low_precision(reason="bf16 partials, fp32 accum"):
                nc.gpsimd.tensor_reduce(
                    out=p2[0:1, SUB:SUB + 1], in_=vprod[:],
                    axis=mybir.AxisListType.XYZWC, op=mybir.AluOpType.add,
                )

            # s partials: W^T (m-major) * x, reduced over chunks -> [P, 64]
            gW_mj = gW[:].rearrange("p (m j) -> p m j", j=NCHUNK)
            sprod = sb.tile([P, SUB, NCHUNK], BF16, tag="sprod")
            nc.vector.tensor_tensor(
                out=sprod[:], in0=gW_mj,
                in1=x_bf[:, None, :].to_broadcast([P, SUB, NCHUNK]),
                op=mybir.AluOpType.mult,
            )
            spart = sb.tile([P, SUB], BF16, tag="spart")
            with nc.allow_low_precision(reason="bf16 partials, fp32 accum"):
                nc.vector.tensor_reduce(
                    out=spart[:], in_=sprod[:], axis=mybir.AxisListType.X,
                    op=mybir.AluOpType.add,
                )

            # s broadcast to all partitions: sb_ps = ones^T @ spart
            sb_ps = ps.tile([P, SUB], F32, tag="sbps")
            nc.tensor.matmul(sb_ps[:], lhsT=ones_bf[:], rhs=spart[:],
                             start=True, stop=True)

            # c = E @ s: E natural [p, ck, m] * s broadcast (read straight
            # from PSUM), reduce over m
            gE_cm = gE[:, 0:E_COLS].rearrange("p (c m) -> p c m", m=SUB)
            cprod = sb.tile([P, ACHUNK, SUB], BF16, tag="cprod")
            nc.vector.tensor_tensor(
                out=cprod[:], in0=gE_cm,
                in1=sb_ps[:, None, :].to_broadcast([P, ACHUNK, SUB]),
                op=mybir.AluOpType.mult,
            )
            c_sb = sb.tile([P, ACHUNK], F32, tag="csb")
            nc.vector.tensor_reduce(
                out=c_sb[:], in_=cprod[:], axis=mybir.AxisListType.X,
                op=mybir.AluOpType.add,
            )

            # leaky relu with offset: c * (0.01 + 0.99*(c >= off))
            cmask = sb.tile([P, ACHUNK], F32, tag="cmask")
            nc.vector.tensor_scalar(
                out=cmask[:], in0=c_sb[:], scalar1=OFFSET, scalar2=None,
                op0=mybir.AluOpType.is_ge,
            )
            cfac = sb.tile([P, ACHUNK], F32, tag="cfac")
            nc.vector.tensor_scalar(
                out=cfac[:], in0=cmask[:], scalar1=0.99, scalar2=0.01,
                op0=mybir.AluOpType.mult, op1=mybir.AluOpType.add,
            )
            c_relu = sb.tile([P, ACHUNK], BF16, tag="crelu")
            nc.vector.tensor_tensor(
                out=c_relu[:], in0=c_sb[:], in1=cfac[:],
                op=mybir.AluOpType.mult,
            )

            # d partials: E natural m-major block [p, m, ck] * c, reduce
            # over ck (contiguous in0)
            gE_mc = gE[:, MC_OFF:MC_OFF + E_COLS].rearrange(
                "p (m c) -> p m c", c=ACHUNK)
            dprod = sb.tile([P, SUB, ACHUNK], BF16, tag="dprod")
            nc.vector.tensor_tensor(
                out=dprod[:], in0=gE_mc,
                in1=c_relu[:, None, :].to_broadcast([P, SUB, ACHUNK]),
                op=mybir.AluOpType.mult,
            )
            with nc.allow_low_precision(reason="bf16 partials, fp32 accum"):
                nc.vector.tensor_reduce(
                    out=p2[:, 0:SUB], in_=dprod[:],
                    axis=mybir.AxisListType.X, op=mybir.AluOpType.add,
                )

            # broadcast [d | v]: bb = ones^T @ p2  (col 64 sums the gate
            # partials at the same time)
            bb_ps = ps.tile([P, SUB + 1], F32, tag="bb")
            nc.tensor.matmul(bb_ps[:], lhsT=ones_bf[:], rhs=p2[:],
                             start=True, stop=True)

            # top-level gate: v = v_raw * (v_raw >= off)  (slope 0.0)
            gmask = sb.tile([P, 1], F32, tag="gmask")
            nc.vector.tensor_scalar(
                out=gmask[:], in0=bb_ps[:, SUB:SUB + 1], scalar1=OFFSET,
                scalar2=None, op0=mybir.AluOpType.is_ge,
            )
            gv = sb.tile([P, 1], F32, tag="gv")
            nc.vector.tensor_tensor(
                out=gv[:], in0=bb_ps[:, SUB:SUB + 1], in1=gmask[:],
                op=mybir.AluOpType.mult,
            )

            # recon: W^T own-half jj-major (contiguous) * d broadcast (read
            # straight from PSUM), reduce over m; one output DMA (a second
            # DMA's completion straggler costs more than the overlap wins)
            gR_jm = gR[:].rearrange("p (j m) -> p j m", m=SUB)
            final = sb.tile([P, HALF], F32, tag="final")
            for pi, (j0, j1) in enumerate([(0, RA), (RA, HALF)]):
                jn = j1 - j0
                rprod = sb.tile([P, jn, SUB], BF16, tag=f"rprod{pi}")
                nc.vector.tensor_tensor(
                    out=rprod[:], in0=gR_jm[:, j0:j1, :],
                    in1=bb_ps[:, None, 0:SUB].to_broadcast([P, jn, SUB]),
                    op=mybir.AluOpType.mult,
                )
                recon = sb.tile([P, jn], F32, tag=f"recon{pi}")
                nc.vector.tensor_reduce(
                    out=recon[:], in_=rprod[:], axis=mybir.AxisListType.X,
                    op=mybir.AluOpType.add,
                )
                nc.vector.scalar_tensor_tensor(
                    out=final[:, j0:j1],
                    in0=gE[:, R_OFF + j0:R_OFF + j1],
                    scalar=gv[:],
                    in1=recon[:],
                    op0=mybir.AluOpType.mult, op1=mybir.AluOpType.add,
                )
            nc.sync.dma_start(out_d[:], final[:])

    nc.compile()
    return nc


def _chunk_order(h):
    """Chunk visit order for core-half h: own half first."""
    own = list(range(h * HALF, (h + 1) * HALF))
    other = list(range((1 - h) * HALF, (2 - h) * HALF))
    return own + other


def _host_prep(x, enc_top, W_down, encoder_weights):
    """Build per-core-half input tables (pure layout transforms)."""
    x = np.asarray(x, np.float32)
    enc_top = np.asarray(enc_top, np.float32)
    W_down = np.asarray(W_down, np.float32)
    E = np.asarray(encoder_weights, np.float32)

    # E natural blocks: ck-major [g, p, ck*64+m] and m-major
    # [g, p, m*4+ck], both = E[g, ck*128+p, m]
    Enat = E.reshape(NE, ACHUNK, P, SUB)
    encnat_cm = np.ascontiguousarray(
        Enat.transpose(0, 2, 1, 3)
    ).reshape(NE, P, E_COLS).astype(ml_dtypes.bfloat16)
    encnat_mc = np.ascontiguousarray(
        Enat.transpose(0, 2, 3, 1)
    ).reshape(NE, P, E_COLS).astype(ml_dtypes.bfloat16)

    Wr = W_down.reshape(NE, SUB, NCHUNK, P)          # [g, m, j, p]
    Er = enc_top.reshape(NE, NCHUNK, P)              # [g, j, p]

    per_half = {}
    for h in (0, 1):
        order = _chunk_order(h)
        # W^T m-major: [g, p, m*18+jj] = W[g, m, order[jj]*128+p]
        tabW = np.ascontiguousarray(
            Wr[:, :, order, :].transpose(0, 3, 1, 2)  # [g, p, m, jj]
        ).reshape(NE, P, W_COLS).astype(ml_dtypes.bfloat16)
        # W^T own-half jj-major: [g, p, jj*64+m]
        tabR = np.ascontiguousarray(
            Wr[:, :, order[:HALF], :].transpose(0, 3, 2, 1)  # [g, p, j, m]
        ).reshape(NE, P, WR_COLS).astype(ml_dtypes.bfloat16)
        encrow = (
            Er[:, order, :].transpose(0, 2, 1)        # [g, p, jj]
        ).astype(ml_dtypes.bfloat16)
        tabE = np.concatenate([encnat_cm, encnat_mc, encrow], axis=2)

        x_pm = np.ascontiguousarray(
            x.reshape(NCHUNK, P)[order, :].T)          # [p, jj]
        encf8 = np.ascontiguousarray(
            Er[:, order, :].transpose(2, 1, 0)         # [p, jj, g]
        ).astype(ml_dtypes.float8_e4m3)
        per_half[h] = dict(
            tabw=tabW,
            tabr=tabR,
            tabe=tabE,
            xbf=x_pm.astype(ml_dtypes.bfloat16),
            xq8=x_pm.astype(ml_dtypes.float8_e4m3),
            encf8=encf8,
        )

    in_maps = []
    for c in range(N_CORES):
        h, slot = c // 4, c % 4
        ph = per_half[h]
        blob = np.zeros((P, G0_COLS), np.uint8)
        blob[:, 0:G0_BYTES] = (
            ph["encf8"][:, 0:G0_CHUNKS, :].reshape(P, G0_BYTES)
            .view(np.uint8))
        blob[:, XBF_OFF:XBF_OFF + 36] = ph["xbf"].view(np.uint8)
        blob[:, XQ8_OFF:XQ8_OFF + 18] = ph["xq8"].view(np.uint8)
        ohu = np.zeros(8, np.uint32)
        ohu[slot] = 1
        blob[:, OHU_OFF:OHU_OFF + 32] = ohu.view(np.uint8)[None, :]
        in_maps.append({
            "tabw": ph["tabw"],
            "tabr": ph["tabr"],
            "tabe": ph["tabe"],
            "encg0": blob,
            "encq": np.ascontiguousarray(ph["encf8"][:, G0_CHUNKS:, :]),
        })
    return in_maps


def _assemble(results):
    out = np.zeros(IN_DIM, np.float32).reshape(NCHUNK, P)
    for c in range(N_CORES):
        h = c // 4
        own = _chunk_order(h)[:HALF]
        out[own, :] += results[c]["out"].T
    return out.reshape(IN_DIM)


_NC_CACHE = {}
LAST_RESULT = {}


def kernel(x, enc_top, W_down, encoder_weights):
    in_maps = _host_prep(x, enc_top, W_down, encoder_weights)
    if "nc" not in _NC_CACHE:
        _NC_CACHE["nc"] = build_program()
    nc = _NC_CACHE["nc"]

    if os.environ.get("BASS_SIM") == "1":
        from concourse.bass_interp import CoreSim
        sim_cores = os.environ.get("BASS_SIM_CORES")
        cores = (
            [int(t) for t in sim_cores.split(",")] if sim_cores
            else range(N_CORES)
        )
        results = [None] * N_CORES
        for c in cores:
            nc_c = build_program()
            sim = CoreSim(nc_c)
            for name, arr in in_maps[c].items():
                sim.tensor(name)[:] = arr
            sim.simulate()
            results[c] = {"out": np.array(sim.tensor("out"))}
        for c in range(N_CORES):
            if results[c] is None:
                results[c] = {"out": np.zeros((P, HALF), np.float32)}
        return _assemble(results)

    trace = os.environ.get("BASS_TRACE") == "1"
    if trace:
        _ensure_trace_hook()
    res = run_bass_kernel_spmd(
        nc, in_maps, core_ids=list(range(N_CORES)),
        trace=trace,
    )
    LAST_RESULT["res"] = res
    return _assemble(res.results)


def _ensure_trace_hook():
    """Install the axon NTFF profile hook if antenv.axon_hooks is absent."""
    try:
        from antenv.axon_hooks import get_axon_ntff_profile_hook  # noqa
        return
    except ImportError:
        pass
    import sys
    import types
    try:
        from trn_agent_boot.trn_boot import _ntff_profile_via_ctypes
    except ImportError:
        return
    hook = _ntff_profile_via_ctypes("/opt/axon/libaxon_pjrt.so")
    mod = types.ModuleType("antenv.axon_hooks")
    mod._hook = hook
    mod.get_axon_ntff_profile_hook = lambda: mod._hook
    mod.set_axon_ntff_profile_hook = lambda h: setattr(mod, "_hook", h)
    import antenv
    sys.modules["antenv.axon_hooks"] = mod
    antenv.axon_hooks = mod


if __name__ == "__main__":
    nc = build_program()
    print("program built ok")


# revision 28
# speedup vs baseline: 1.2534x; 1.0261x over previous
"""Trainium2 Bass kernel for single-token MoE routing (nn_MixtureOfExperts_v2).

Problem:
    x [2304]; enc_top [256, 2304]; W_down [256, 64, 2304]; encoder_weights
    [256, 512, 64].
    codes = relu_offset(enc_top @ x)           (slope 0.0, offset 1/48)
    top4 values/indices of codes
    per selected expert i (gate v):
        s = W_down[i] @ x                      [64]
        c = relu_offset(E[i] @ s, slope 0.01)  [512]
        d = E[i]^T @ c                         [64]
        recon += W_down[i]^T @ d               [2304]
        recon += v * enc_top[i]
    output = recon                             [2304]

Distribution (8 cores, no collectives):
    Every core loads a replicated fp8 transposed copy of enc_top, computes
    all 256 codes on the PE, and runs top-4 on the vector engine
    (max_with_indices), so all cores agree on the routing.  Core c then
    processes selected slot (c % 4) alone: it gathers that expert's weights
    (bf16) with two register-offset direct DMAs and runs the expert
    pipeline.  Cores c and c+4 process the same slot but emit complementary
    halves of the 2304-dim reconstruction (the per-core tables are built
    with the core's half of the input-dim chunks first, so the program is
    identical across cores - pure SPMD with per-core constants).  The host
    sums the 8 partial outputs (the cross-core reduction is a plain "+"
    done during unsharding).

Expert pipeline dataflow (v2): the skinny matvecs (s = W @ x and
d = E^T @ c) run on the vector engine as broadcast-multiply + reduce over
the free dim, leaving only cross-partition sums / broadcasts to the PE
(two matmuls against a constant all-ones weight).  This avoids the
~125ns/matmul LDWEIGHTS floor of a PE-side chunk loop and is insensitive
to the HAM clock throttle.  All gathered tables are bf16; routing runs in
fp8 (selection-only; the gate value is recomputed from bf16 tables).
"""

import os

import numpy as np
import ml_dtypes

import concourse.bacc as bacc
import concourse.bass as bass
import concourse.mybir as mybir
import concourse.tile as tile
from concourse.bass_utils import run_bass_kernel_spmd

# ---- problem constants (hardcoded per harness contract) ----
IN_DIM = 2304
SUB = 64
ATOMS = 512
NE = 256
K = 4
P = 128
NCHUNK = IN_DIM // P          # 18 chunks of 128 along input dim
HALF = NCHUNK // 2            # 9 chunks per core-half
ACHUNK = ATOMS // P           # 4 chunks of 128 along atoms
N_CORES = 8

W_COLS = SUB * NCHUNK         # 1152: W^T block, m-major (jj innermost)
WR_COLS = HALF * SUB          # 576:  W^T own-half block, jj-major (m inner)
E_COLS = ACHUNK * SUB         # 256:  E natural block, ck-major (m inner)
MC_OFF = E_COLS               # 256:  E natural block, m-major (ck inner)
R_OFF = 2 * E_COLS            # 512:  enc_top row (chunk-major)
R_COLS = NCHUNK               # 18
TABE_COLS = R_OFF + R_COLS    # 530
RA = 5                        # recon first-half chunks (second: HALF-RA)

# enc chunk groups per DMA after the merged first group: (queue, nchunks).
# All enc traffic stays on the sync queue: a DMA's completion semaphore has
# been observed to lag its last byte by 1-2.5us when the other queue also
# has traffic in flight.
G0_CHUNKS = 2                 # chunks merged with the consts in encg0
ENC_GROUPS = [("sync", 8), ("sync", 8)]
G0_BYTES = G0_CHUNKS * NE     # 512
XBF_OFF = G0_BYTES            # 512: x bf16 (36 bytes)
XQ8_OFF = XBF_OFF + 36        # 548: x fp8 (18 bytes)
OHU_OFF = 568                 # one-hot u32 (32 bytes), 4-aligned
G0_COLS = 600
N_PREWARM = int(os.environ.get("KERNEL_PREWARM_MMS", "22"))
# junk matmuls interleaved after the g0 codes matmuls: keep the PE busy
# through the g1-semaphore wait so HAM un-throttles to 2.4 GHz before the
# bulk of the codes matmuls
N_MIDWARM = int(os.environ.get("KERNEL_MIDWARM_MMS", "12"))

OFFSET = float(np.float32(1.0) / np.float32(48.0))  # 1/sqrt(2304), fp32

F32 = mybir.dt.float32
BF16 = mybir.dt.bfloat16
F8 = mybir.dt.float8e4
I32 = mybir.dt.int32
U32 = mybir.dt.uint32


def build_program():
    nc = bacc.Bacc("TRN2", target_bir_lowering=False, debug=False,
                   enable_partition_id=False)

    tabW = nc.dram_tensor("tabw", [NE, P, W_COLS], BF16,
                          kind="ExternalInput")
    tabR = nc.dram_tensor("tabr", [NE, P, WR_COLS], BF16,
                          kind="ExternalInput")
    tabE = nc.dram_tensor("tabe", [NE, P, TABE_COLS], BF16,
                          kind="ExternalInput")
    # merged first group: enc chunks 0:2 (fp8) + x bf16 + x fp8 + one-hot,
    # one DMA -> one semaphore gating the first codes matmuls
    encg0 = nc.dram_tensor("encg0", [P, G0_COLS], mybir.dt.uint8,
                           kind="ExternalInput")
    encq = nc.dram_tensor("encq", [P, NCHUNK - G0_CHUNKS, NE], F8,
                          kind="ExternalInput")
    out_d = nc.dram_tensor("out", [P, HALF], F32, kind="ExternalOutput")

    with tile.TileContext(nc) as tc:
        with (
            tc.tile_pool(name="sb", bufs=1) as sb,
            tc.tile_pool(name="enc", bufs=1) as encp,
            tc.tile_pool(name="ps", bufs=1, space="PSUM") as ps,
        ):
            # ---- phase A: codes = enc_top @ x (fp8, PE) ----
            g0t = sb.tile([P, G0_COLS], mybir.dt.uint8, tag="encg0")
            nc.sync.dma_start(g0t[:], encg0[:])
            enc0 = g0t[:, 0:G0_BYTES].bitcast(F8).rearrange(
                "p (g e) -> p g e", e=NE)
            x_bf = g0t[:, XBF_OFF:XBF_OFF + 36].bitcast(BF16)   # [P, 18]
            x_q8 = g0t[:, XQ8_OFF:XQ8_OFF + 18].bitcast(F8)     # [P, 18]
            ohu = g0t[0:1, OHU_OFF:OHU_OFF + 32].bitcast(U32)   # [1, 8]
            enc_ts = [(enc0, 0, G0_CHUNKS)]
            g0 = G0_CHUNKS
            for gi, (q, gn) in enumerate(ENC_GROUPS):
                enc_t = encp.tile([P, gn, NE], F8, tag=f"enc{gi}")
                eng = nc.sync if q == "sync" else nc.scalar
                eng.dma_start(enc_t[:], encq[:, g0 - G0_CHUNKS:
                                             g0 - G0_CHUNKS + gn, :])
                enc_ts.append((enc_t, g0, gn))
                g0 += gn

            # on-device constants
            ones_bf = sb.tile([P, P], BF16, tag="onesbf")
            nc.vector.memset(ones_bf[:], 1.0)

            # ---- PE pre-warm: matmuls on the ones tile while the first
            # enc-group DMA is in flight, so HAM un-throttles the PE to
            # 2.4 GHz before the codes matmuls start ----
            junk_ps = ps.tile([1, NE], F32, tag="junk")
            if N_PREWARM:
                for w in range(N_PREWARM):
                    nc.tensor.matmul(
                        junk_ps[:, 0:P],
                        lhsT=ones_bf[:, 0:1],
                        rhs=ones_bf[:],
                        start=(w == 0),
                        stop=(w == N_PREWARM - 1),
                    )

            codes_ps = ps.tile([1, NE], F32, tag="codes")
            for enc_t, g0, gn in enc_ts:
                for jo in range(gn):
                    jj = g0 + jo
                    nc.tensor.matmul(
                        codes_ps[:],
                        lhsT=x_q8[:, jj:jj + 1],
                        rhs=enc_t[:, jo, :],
                        start=(jj == 0),
                        stop=(jj == NCHUNK - 1),
                    )
                if g0 == 0 and N_MIDWARM:
                    for w in range(N_MIDWARM):
                        nc.tensor.matmul(
                            junk_ps[:, 0:P],
                            lhsT=ones_bf[:, 0:1],
                            rhs=ones_bf[:],
                            start=(w == 0),
                            stop=(w == N_MIDWARM - 1),
                        )

            # ---- phase B: top-k (max8 on DVE, reading PSUM) + slot pick ----
            vals = sb.tile([1, 8], F32, tag="vals")
            idxs = sb.tile([1, 8], U32, tag="idxs")
            nc.vector.max_with_indices(vals[:], idxs[:], codes_ps[:])
            scr8 = sb.tile([1, 8], U32, tag="scr8")
            nc.vector.tensor_tensor(
                out=scr8[:], in0=idxs[:], in1=ohu,
                op=mybir.AluOpType.mult,
            )
            isel_u = sb.tile([1, 1], U32, tag="iselu")
            with nc.allow_low_precision(
                    reason="one-hot dot on u32 indices; exact"):
                nc.vector.tensor_reduce(
                    out=isel_u[:], in_=scr8[:], axis=mybir.AxisListType.X,
                    op=mybir.AluOpType.add,
                )
            val = nc.values_load(
                isel_u[:],
                engines={mybir.EngineType.SP, mybir.EngineType.Activation},
                min_val=0, max_val=NE - 1, skip_runtime_bounds_check=True,
            )

            # ---- phase C: gather this slot's expert blocks with
            # register-offset direct DMAs (HWDGE).  W (m-major, for s) and
            # the recon block (jj-major own half) are separate DMAs so the
            # s partials can start before the recon block lands. ----
            gW = sb.tile([P, W_COLS], BF16, tag="gw")
            nc.sync.dma_start(gW[:], tabW[bass.ds(val, 1), :, :])
            gE = sb.tile([P, TABE_COLS], BF16, tag="ge")
            nc.scalar.dma_start(gE[:], tabE[bass.ds(val, 1), :, :])
            gR = sb.tile([P, WR_COLS], BF16, tag="gr")
            nc.scalar.dma_start(gR[:], tabR[bass.ds(val, 1), :, :])

            # ---- phase D: expert pipeline (bf16 DVE/PE hybrid) ----
            # per-partition partials of d and of the gate dot, summed and
            # broadcast by one ones-weight matmul: bb = ones^T @ [d | v]
            p2 = sb.tile([P, SUB + 1], BF16, tag="p2")

            # gate dot: v_raw = sum(enc_row * x).  Runs entirely on the
            # otherwise-idle gpsimd engine (its reduce sums across
            # partitions too), so it never blocks the DVE chain.  v_raw
            # lands in p2[0, 64] with the rest of that column zeroed; the
            # bb matmul's column sum then broadcasts it to all partitions.
            nc.gpsimd.memset(p2[:, SUB:SUB + 1], 0.0)
            vprod = sb.tile([P, NCHUNK], BF16, tag="vprod")
            nc.gpsimd.tensor_tensor(
                out=vprod[:], in0=gE[:, R_OFF:R_OFF + NCHUNK], in1=x_bf,
                op=mybir.AluOpType.mult,
            )
            with nc.allow_low_precision(reason="bf16 partials, fp32 accum"):
                nc.gpsimd.tensor_reduce(
                    out=p2[0:1, SUB:SUB + 1], in_=vprod[:],
                    axis=mybir.AxisListType.XYZWC, op=mybir.AluOpType.add,
                )

            # s partials: W^T (m-major) * x, reduced over chunks -> [P, 64]
            gW_mj = gW[:].rearrange("p (m j) -> p m j", j=NCHUNK)
            sprod = sb.tile([P, SUB, NCHUNK], BF16, tag="sprod")
            nc.vector.tensor_tensor(
                out=sprod[:], in0=gW_mj,
                in1=x_bf[:, None, :].to_broadcast([P, SUB, NCHUNK]),
                op=mybir.AluOpType.mult,
            )
            spart = sb.tile([P, SUB], BF16, tag="spart")
            with nc.allow_low_precision(reason="bf16 partials, fp32 accum"):
                nc.vector.tensor_reduce(
                    out=spart[:], in_=sprod[:], axis=mybir.AxisListType.X,
                    op=mybir.AluOpType.add,
                )

            # s broadcast to all partitions: sb_ps = ones^T @ spart
            sb_ps = ps.tile([P, SUB], F32, tag="sbps")
            nc.tensor.matmul(sb_ps[:], lhsT=ones_bf[:], rhs=spart[:],
                             start=True, stop=True)

            # c = E @ s: E natural [p, ck, m] * s broadcast (read straight
            # from PSUM), reduce over m
            gE_cm = gE[:, 0:E_COLS].rearrange("p (c m) -> p c m", m=SUB)
            cprod = sb.tile([P, ACHUNK, SUB], BF16, tag="cprod")
            nc.vector.tensor_tensor(
                out=cprod[:], in0=gE_cm,
                in1=sb_ps[:, None, :].to_broadcast([P, ACHUNK, SUB]),
                op=mybir.AluOpType.mult,
            )
            c_sb = sb.tile([P, ACHUNK], F32, tag="csb")
            nc.vector.tensor_reduce(
                out=c_sb[:], in_=cprod[:], axis=mybir.AxisListType.X,
                op=mybir.AluOpType.add,
            )

            # leaky relu with offset: c * (0.01 + 0.99*(c >= off))
            cmask = sb.tile([P, ACHUNK], F32, tag="cmask")
            nc.vector.tensor_scalar(
                out=cmask[:], in0=c_sb[:], scalar1=OFFSET, scalar2=None,
                op0=mybir.AluOpType.is_ge,
            )
            cfac = sb.tile([P, ACHUNK], F32, tag="cfac")
            nc.vector.tensor_scalar(
                out=cfac[:], in0=cmask[:], scalar1=0.99, scalar2=0.01,
                op0=mybir.AluOpType.mult, op1=mybir.AluOpType.add,
            )
            c_relu = sb.tile([P, ACHUNK], BF16, tag="crelu")
            nc.vector.tensor_tensor(
                out=c_relu[:], in0=c_sb[:], in1=cfac[:],
                op=mybir.AluOpType.mult,
            )

            # d partials: E natural m-major block [p, m, ck] * c, reduce
            # over ck (contiguous in0)
            gE_mc = gE[:, MC_OFF:MC_OFF + E_COLS].rearrange(
                "p (m c) -> p m c", c=ACHUNK)
            dprod = sb.tile([P, SUB, ACHUNK], BF16, tag="dprod")
            nc.vector.tensor_tensor(
                out=dprod[:], in0=gE_mc,
                in1=c_relu[:, None, :].to_broadcast([P, SUB, ACHUNK]),
                op=mybir.AluOpType.mult,
            )
            with nc.allow_low_precision(reason="bf16 partials, fp32 accum"):
                nc.vector.tensor_reduce(
                    out=p2[:, 0:SUB], in_=dprod[:],
                    axis=mybir.AxisListType.X, op=mybir.AluOpType.add,
                )

            # broadcast [d | v]: bb = ones^T @ p2  (col 64 sums the gate
            # partials at the same time)
            bb_ps = ps.tile([P, SUB + 1], F32, tag="bb")
            nc.tensor.matmul(bb_ps[:], lhsT=ones_bf[:], rhs=p2[:],
                             start=True, stop=True)

            # top-level gate: v = v_raw * (v_raw >= off)  (slope 0.0)
            gmask = sb.tile([P, 1], F32, tag="gmask")
            nc.vector.tensor_scalar(
                out=gmask[:], in0=bb_ps[:, SUB:SUB + 1], scalar1=OFFSET,
                scalar2=None, op0=mybir.AluOpType.is_ge,
            )
            gv = sb.tile([P, 1], F32, tag="gv")
            nc.vector.tensor_tensor(
                out=gv[:], in0=bb_ps[:, SUB:SUB + 1], in1=gmask[:],
                op=mybir.AluOpType.mult,
            )

            # recon: W^T own-half jj-major (contiguous) * d broadcast (read
            # straight from PSUM), reduce over m; one output DMA (a second
            # DMA's completion straggler costs more than the overlap wins)
            gR_jm = gR[:].rearrange("p (j m) -> p j m", m=SUB)
            final = sb.tile([P, HALF], F32, tag="final")
            for pi, (j0, j1) in enumerate([(0, RA), (RA, HALF)]):
                jn = j1 - j0
                rprod = sb.tile([P, jn, SUB], BF16, tag=f"rprod{pi}")
                nc.vector.tensor_tensor(
                    out=rprod[:], in0=gR_jm[:, j0:j1, :],
                    in1=bb_ps[:, None, 0:SUB].to_broadcast([P, jn, SUB]),
                    op=mybir.AluOpType.mult,
                )
                recon = sb.tile([P, jn], F32, tag=f"recon{pi}")
                nc.vector.tensor_reduce(
                    out=recon[:], in_=rprod[:], axis=mybir.AxisListType.X,
                    op=mybir.AluOpType.add,
                )
                nc.vector.scalar_tensor_tensor(
                    out=final[:, j0:j1],
                    in0=gE[:, R_OFF + j0:R_OFF + j1],
                    scalar=gv[:],
                    in1=recon[:],
                    op0=mybir.AluOpType.mult, op1=mybir.AluOpType.add,
                )
            nc.scalar.dma_start(out_d[:], final[:])

    nc.compile()
    return nc


def _chunk_order(h):
    """Chunk visit order for core-half h: own half first."""
    own = list(range(h * HALF, (h + 1) * HALF))
    other = list(range((1 - h) * HALF, (2 - h) * HALF))
    return own + other


def _host_prep(x, enc_top, W_down, encoder_weights):
    """Build per-core-half input tables (pure layout transforms)."""
    x = np.asarray(x, np.float32)
    enc_top = np.asarray(enc_top, np.float32)
    W_down = np.asarray(W_down, np.float32)
    E = np.asarray(encoder_weights, np.float32)

    # E natural blocks: ck-major [g, p, ck*64+m] and m-major
    # [g, p, m*4+ck], both = E[g, ck*128+p, m]
    Enat = E.reshape(NE, ACHUNK, P, SUB)
    encnat_cm = np.ascontiguousarray(
        Enat.transpose(0, 2, 1, 3)
    ).reshape(NE, P, E_COLS).astype(ml_dtypes.bfloat16)
    encnat_mc = np.ascontiguousarray(
        Enat.transpose(0, 2, 3, 1)
    ).reshape(NE, P, E_COLS).astype(ml_dtypes.bfloat16)

    Wr = W_down.reshape(NE, SUB, NCHUNK, P)          # [g, m, j, p]
    Er = enc_top.reshape(NE, NCHUNK, P)              # [g, j, p]

    per_half = {}
    for h in (0, 1):
        order = _chunk_order(h)
        # W^T m-major: [g, p, m*18+jj] = W[g, m, order[jj]*128+p]
        tabW = np.ascontiguousarray(
            Wr[:, :, order, :].transpose(0, 3, 1, 2)  # [g, p, m, jj]
        ).reshape(NE, P, W_COLS).astype(ml_dtypes.bfloat16)
        # W^T own-half jj-major: [g, p, jj*64+m]
        tabR = np.ascontiguousarray(
            Wr[:, :, order[:HALF], :].transpose(0, 3, 2, 1)  # [g, p, j, m]
        ).reshape(NE, P, WR_COLS).astype(ml_dtypes.bfloat16)
        encrow = (
            Er[:, order, :].transpose(0, 2, 1)        # [g, p, jj]
        ).astype(ml_dtypes.bfloat16)
        tabE = np.concatenate([encnat_cm, encnat_mc, encrow], axis=2)

        x_pm = np.ascontiguousarray(
            x.reshape(NCHUNK, P)[order, :].T)          # [p, jj]
        encf8 = np.ascontiguousarray(
            Er[:, order, :].transpose(2, 1, 0)         # [p, jj, g]
        ).astype(ml_dtypes.float8_e4m3)
        per_half[h] = dict(
            tabw=tabW,
            tabr=tabR,
            tabe=tabE,
            xbf=x_pm.astype(ml_dtypes.bfloat16),
            xq8=x_pm.astype(ml_dtypes.float8_e4m3),
            encf8=encf8,
        )

    in_maps = []
    for c in range(N_CORES):
        h, slot = c // 4, c % 4
        ph = per_half[h]
        blob = np.zeros((P, G0_COLS), np.uint8)
        blob[:, 0:G0_BYTES] = (
            ph["encf8"][:, 0:G0_CHUNKS, :].reshape(P, G0_BYTES)
            .view(np.uint8))
        blob[:, XBF_OFF:XBF_OFF + 36] = ph["xbf"].view(np.uint8)
        blob[:, XQ8_OFF:XQ8_OFF + 18] = ph["xq8"].view(np.uint8)
        ohu = np.zeros(8, np.uint32)
        ohu[slot] = 1
        blob[:, OHU_OFF:OHU_OFF + 32] = ohu.view(np.uint8)[None, :]
        in_maps.append({
            "tabw": ph["tabw"],
            "tabr": ph["tabr"],
            "tabe": ph["tabe"],
            "encg0": blob,
            "encq": np.ascontiguousarray(ph["encf8"][:, G0_CHUNKS:, :]),
        })
    return in_maps


def _assemble(results):
    out = np.zeros(IN_DIM, np.float32).reshape(NCHUNK, P)
    for c in range(N_CORES):
        h = c // 4
        own = _chunk_order(h)[:HALF]
        out[own, :] += results[c]["out"].T
    return out.reshape(IN_DIM)


_NC_CACHE = {}
LAST_RESULT = {}


def kernel(x, enc_top, W_down, encoder_weights):
    in_maps = _host_prep(x, enc_top, W_down, encoder_weights)
    if "nc" not in _NC_CACHE:
        _NC_CACHE["nc"] = build_program()
    nc = _NC_CACHE["nc"]

    if os.environ.get("BASS_SIM") == "1":
        from concourse.bass_interp import CoreSim
        sim_cores = os.environ.get("BASS_SIM_CORES")
        cores = (
            [int(t) for t in sim_cores.split(",")] if sim_cores
            else range(N_CORES)
        )
        results = [None] * N_CORES
        for c in cores:
            nc_c = build_program()
            sim = CoreSim(nc_c)
            for name, arr in in_maps[c].items():
                sim.tensor(name)[:] = arr
            sim.simulate()
            results[c] = {"out": np.array(sim.tensor("out"))}
        for c in range(N_CORES):
            if results[c] is None:
                results[c] = {"out": np.zeros((P, HALF), np.float32)}
        return _assemble(results)

    trace = os.environ.get("BASS_TRACE") == "1"
    if trace:
        _ensure_trace_hook()
    res = run_bass_kernel_spmd(
        nc, in_maps, core_ids=list(range(N_CORES)),
        trace=trace,
    )
    LAST_RESULT["res"] = res
    return _assemble(res.results)


def _ensure_trace_hook():
    """Install the axon NTFF profile hook if antenv.axon_hooks is absent."""
    try:
        from antenv.axon_hooks import get_axon_ntff_profile_hook  # noqa
        return
    except ImportError:
        pass
    import sys
    import types
    try:
        from trn_agent_boot.trn_boot import _ntff_profile_via_ctypes
    except ImportError:
        return
    hook = _ntff_profile_via_ctypes("/opt/axon/libaxon_pjrt.so")
    mod = types.ModuleType("antenv.axon_hooks")
    mod._hook = hook
    mod.get_axon_ntff_profile_hook = lambda: mod._hook
    mod.set_axon_ntff_profile_hook = lambda h: setattr(mod, "_hook", h)
    import antenv
    sys.modules["antenv.axon_hooks"] = mod
    antenv.axon_hooks = mod


if __name__ == "__main__":
    nc = build_program()
    print("program built ok")
